# revision 22
# baseline (speedup 1.0000x reference)
"""Trainium2 Bass kernel for masked-GRU + residual + LayerNorm.

Problem: N=128 sequences of length L=512, hidden H=512.
  gx = x @ W_ih.T + b_ih            (precomputable input projection)
  per step l: hc = h * (1-is_initial[l]); gh = hc @ W_hh.T + b_hh
    r = sig(gx_r+gh_r); z = sig(gx_z+gh_z); n = tanh(gx_n + r*gh_n)
    h = (1-z)*n + z*hc
  out = LayerNorm(seq + x) * gamma + beta;  h_exp = broadcast(h_last)

Strategy:
  * Data parallel: 16 batch rows per core (8 cores).
  * Sequence-chunk parallel: each L=512 sequence is split into C=16
    chunks of 32 steps, made exact by an R-step warm-up (state entering
    a chunk only depends on inputs back to the latest reset; R covers
    the max reset gap, checked at runtime). Chunk 0 injects true h0.
  * fp16 end-to-end: matmul operands, state, gates, outputs (validated
    vs f64 reference: ~1.6e-3 max rel err).
  * Per-step whh matmuls in PSUM; wih prefilled one step ahead (r and
    gx_n groups) to keep the in-order PE queue busy during gate math.
  * LayerNorm mu/ss matmuls + stats + apply are DEFERRED one step so
    they never stall the in-order Tensor queue on the gate chain.
  * All masks preloaded once (partition-broadcast DMA); x loaded with
    one DMA per step; state init via memset.
"""
import sys

sys.path.insert(0, "/opt/trn_rl_repo")

import numpy as np

import concourse.bass as bass
import concourse.tile as tile
from concourse import bacc, mybir
from concourse.bass_utils import run_bass_kernel_spmd

F32 = mybir.dt.float32
F16 = mybir.dt.float16
AF = mybir.ActivationFunctionType
ALU = mybir.AluOpType

N, L, H = 128, 512, 512
NCORES = 8
NB = N // NCORES          # batch rows per core = 16
C = 16                    # chunks per sequence
KS = L // C               # main steps per chunk = 32
S = NB * C                # columns per core = 256
HT = H // 128             # h partition tiles = 4
BLK = 4                   # LN block (main steps)
NBLK = KS // BLK          # 8


def _bcast_ap(row_ap, parts=128):
    """DRAM row AP -> partition-broadcast AP (step 0 over partitions)."""
    return bass.AP(
        tensor=row_ap.tensor,
        offset=row_ap.offset,
        ap=[[0, parts]] + [list(d) for d in row_ap.ap],
    )


def build_program(R=16, triv_gb=False):
    T = R + KS
    nc = bacc.Bacc("TRN2", target_bir_lowering=False)

    xs_d = nc.declare_dram_parameter("xs", [T, 128, HT * S], F16, isOutput=False)
    ms_d = nc.declare_dram_parameter("ms", [1, T * 2 * S], F16, isOutput=False)
    h0m_d = nc.declare_dram_parameter("h0m", [128, HT * NB], F16, isOutput=False)
    wih_d = nc.declare_dram_parameter("wih", [HT, 128, 3 * H], F16, isOutput=False)
    whh_d = nc.declare_dram_parameter("whh", [HT, 128, 3 * H], F16, isOutput=False)
    brz_d = nc.declare_dram_parameter("brz", [128, 8], F32, isOutput=False)
    bhn_d = nc.declare_dram_parameter("bhn", [128, HT], F32, isOutput=False)
    bin_d = nc.declare_dram_parameter("bin", [128, HT], F32, isOutput=False)
    gam_d = nc.declare_dram_parameter("gam", [128, HT], F32, isOutput=False)
    bet_d = nc.declare_dram_parameter("bet", [128, HT], F32, isOutput=False)
    ones_d = nc.declare_dram_parameter("ones", [128, 1], F16, isOutput=False)

    out_d = nc.declare_dram_parameter("out_st", [HT, 128, KS, S], F16, isOutput=True)
    hl_d = nc.declare_dram_parameter("hlast", [HT, 128, NB], F16, isOutput=True)

    scr = nc.dram_tensor("lnscr", [NBLK, 2048], F16)

    with tile.TileContext(nc) as tc:
        with (
            tc.tile_pool(name="const", bufs=1) as cst,
            tc.tile_pool(name="sb", bufs=1) as sb,
            tc.tile_pool(name="rp", bufs=4, space="PSUM") as rp,
            tc.tile_pool(name="ip", bufs=4, space="PSUM") as ip,
        ):
            # ---- constants (wih first: needed by the t=0 prefill) ----
            wih_sb, whh_sb = [], []
            x0 = sb.tile([128, HT * S], F16, name="xt0", tag="xt", bufs=4)
            nc.sync.dma_start(out=x0, in_=xs_d[0, :, :])
            for k in range(HT):
                w1 = cst.tile([128, 3 * H], F16, name=f"wih_sb{k}", tag=f"wih{k}")
                nc.sync.dma_start(out=w1, in_=wih_d[k, :, :])
                wih_sb.append(w1)
            out_flat = [out_d[k, :, :, :].rearrange("p t s -> p (t s)") for k in range(HT)]

            def xsl(xt, k):
                return xt[:, k * S : (k + 1) * S]

            def ssl(st, k):
                return st[:, k * S : (k + 1) * S]

            def load_x(t):
                xt = sb.tile([128, HT * S], F16, name=f"xt{t}", tag="xt", bufs=4)
                nc.sync.dma_start(out=xt, in_=xs_d[t, :, :])
                return xt

            def prefill_gxn(t, xt):
                # complete psum groups for gx_n of step t (wih only),
                # drained straight to SBUF on ACT with b_in folded in
                gx_ps = [
                    ip.tile([128, 512], F32, name=f"gx{t}_{j}", tag="ip")
                    for j in range(2)
                ]
                for k4 in range(4):
                    j = 8 + k4
                    oap = gx_ps[k4 // 2][:, (k4 % 2) * 256 : (k4 % 2) * 256 + 256]
                    for k in range(HT):
                        nc.tensor.matmul(
                            oap, wih_sb[k][:, j * 128 : (j + 1) * 128], xsl(xt, k),
                            start=(k == 0), stop=(k == HT - 1))
                return gx_ps

            def prefill_r(t, xt):
                # open accumulation groups for the r gate of step t
                r_ps = [
                    rp.tile([128, 256], F32, name=f"r{t}_{j}", tag="rp")
                    for j in range(4)
                ]
                for j in range(4):
                    for k in range(HT):
                        nc.tensor.matmul(
                            r_ps[j], wih_sb[k][:, j * 128 : (j + 1) * 128], xsl(xt, k),
                            start=(k == 0), stop=False)
                return r_ps

            xt = x0
            gx_ps_cur = prefill_gxn(0, xt)
            r_ps = prefill_r(0, xt)

            # ---- remaining constants (loaded while the prefill runs) ----
            for k in range(HT):
                w2 = cst.tile([128, 3 * H], F16, name=f"whh_sb{k}", tag=f"whh{k}")
                nc.sync.dma_start(out=w2, in_=whh_d[k, :, :])
                whh_sb.append(w2)
            h0m_sb = cst.tile([128, HT * NB], F16, name="h0m_sb", tag="h0m")
            nc.sync.dma_start(out=h0m_sb, in_=h0m_d[:, :])
            brz_sb = cst.tile([128, 8], F32, name="brz_sb", tag="brz")
            nc.sync.dma_start(out=brz_sb, in_=brz_d[:, :])
            bhn_sb = cst.tile([128, HT], F32, name="bhn_sb", tag="bhn")
            nc.sync.dma_start(out=bhn_sb, in_=bhn_d[:, :])
            bin_sb = cst.tile([128, HT], F32, name="bin_sb", tag="bin")
            nc.sync.dma_start(out=bin_sb, in_=bin_d[:, :])
            gam_sb = bet_sb = None
            if not triv_gb:
                gam_sb = cst.tile([128, HT], F32, name="gam_sb", tag="gam")
                nc.sync.dma_start(out=gam_sb, in_=gam_d[:, :])
                bet_sb = cst.tile([128, HT], F32, name="bet_sb", tag="bet")
                nc.sync.dma_start(out=bet_sb, in_=bet_d[:, :])
            ones_sb = cst.tile([128, 1], F16, name="ones_sb", tag="ones")
            nc.sync.dma_start(out=ones_sb, in_=ones_d[:, :])
            eps_sb = cst.tile([1, 1], F32, name="eps_sb", tag="eps")
            nc.vector.memset(eps_sb, 1e-5)
            msk = cst.tile([128, T * 2 * S], F16, name="msk", tag="msk")
            MH = 8 * 2 * S
            nc.sync.dma_start(out=msk[:, 0:MH], in_=_bcast_ap(ms_d[0, 0:MH]))
            nc.sync.dma_start(
                out=msk[:, MH : T * 2 * S], in_=_bcast_ap(ms_d[0, MH : T * 2 * S]))

            # ---- initial (zero) state: one [128, HT*S] tile ----
            s_cur = sb.tile([128, HT * S], F16, name="s_init", tag="state", bufs=4)
            nc.vector.memset(s_cur, 0.0)

            y_blk = None
            y2 = None
            pend = None          # (blk, y_blk, y2) finished, LN deferred
            pend_stats = None    # (blk, y_blk, mu_ps, ss_ps) stats deferred
            pend_apply = None    # (blk, y_blk, mu_bc, rs_bc) apply deferred
            for t in range(T):
                main = t >= R
                toff = (t - R) % BLK
                blk = (t - R) // BLK

                # -- gx_n drain (to SBUF, b_in folded): frees its psum
                # slots immediately and fills the engine-idle step start
                gxs2 = [
                    sb.tile([128, 2 * S], F16, name=f"gxs{t}_{p}", tag="gxs", bufs=4)
                    for p in range(2)
                ]
                for k in range(HT):
                    psl = gx_ps_cur[k // 2][:, (k % 2) * 256 : (k % 2) * 256 + 256]
                    nc.scalar.activation(
                        out=gxs2[k // 2][:, (k % 2) * S : (k % 2) * S + S],
                        in_=psl, func=AF.Identity,
                        bias=bin_sb[:, k : k + 1], scale=1.0)

                # ---- deferred LN stats for the block finished last step:
                # mu/ss matmuls go FIRST on the in-order PE queue (their
                # inputs are old => no stall), then the stats chain.
                if pend is not None and (not main or toff == 1):
                    # only the mu/ss matmuls here: they fill the PE lull at
                    # the step start; the stats chain is emitted at the
                    # step BOTTOM so it queues behind the critical gate ops
                    pblk, pyb, py2 = pend
                    mu_ps = [
                        ip.tile([1, 512], F32, name=f"mu{pblk}_{h}", tag="ip")
                        for h in range(2)
                    ]
                    ss_ps = [
                        ip.tile([1, 512], F32, name=f"ss{pblk}_{h}", tag="ip")
                        for h in range(2)
                    ]
                    for half in range(2):
                        for k in range(HT):
                            nc.tensor.matmul(
                                mu_ps[half], ones_sb,
                                pyb[k][:, half * 512 : (half + 1) * 512],
                                start=(k == 0), stop=(k == HT - 1))
                        for k in range(HT):
                            nc.tensor.matmul(
                                ss_ps[half], ones_sb,
                                py2[k][:, half * 512 : (half + 1) * 512],
                                start=(k == 0), stop=(k == HT - 1))
                    pend_stats = (pblk, pyb, mu_ps, ss_ps)
                    pend = None

                if t + 1 < T:
                    xt_nxt = load_x(t + 1)

                # -- close r groups with the recurrent part --
                for k in range(HT):
                    for j in range(4):
                        nc.tensor.matmul(
                            r_ps[j], whh_sb[k][:, j * 128 : (j + 1) * 128],
                            ssl(s_cur, k), start=False, stop=(k == HT - 1))
                # -- gh_n (whh only, complete groups) --
                gh_ps = [
                    ip.tile([128, 512], F32, name=f"gh{t}_{j}", tag="ip")
                    for j in range(2)
                ]
                for k4 in range(4):
                    j = 8 + k4
                    oap = gh_ps[k4 // 2][:, (k4 % 2) * 256 : (k4 % 2) * 256 + 256]
                    for k in range(HT):
                        nc.tensor.matmul(
                            oap, whh_sb[k][:, j * 128 : (j + 1) * 128], ssl(s_cur, k),
                            start=(k == 0), stop=(k == HT - 1))
                # -- z gate (whh + wih complete groups, in-step) --
                z_ps = [
                    ip.tile([128, 512], F32, name=f"z{t}_{j}", tag="ip")
                    for j in range(2)
                ]
                for j4 in range(4):
                    j = 4 + j4
                    oap = z_ps[j4 // 2][:, (j4 % 2) * 256 : (j4 % 2) * 256 + 256]
                    for k in range(HT):
                        nc.tensor.matmul(
                            oap, whh_sb[k][:, j * 128 : (j + 1) * 128], ssl(s_cur, k),
                            start=(k == 0), stop=False)
                    for k in range(HT):
                        nc.tensor.matmul(
                            oap, wih_sb[k][:, j * 128 : (j + 1) * 128], xsl(xt, k),
                            start=False, stop=(k == HT - 1))

                # -- prefill next step (PE stays busy during gate math) --
                if t + 1 < T:
                    gx_ps_nxt = prefill_gxn(t + 1, xt_nxt)
                    r_nxt = prefill_r(t + 1, xt_nxt)

                # -- sigmoids straight from PSUM (bias = b_ih + b_hh) --
                r_t, z_t = [], []
                for k in range(HT):
                    rt = sb.tile([128, S], F16, name=f"rt{t}_{k}", tag="rt", bufs=4)
                    nc.scalar.activation(
                        out=rt, in_=r_ps[k],
                        func=AF.Sigmoid, bias=brz_sb[:, k : k + 1], scale=1.0)
                    r_t.append(rt)
                zt2 = [
                    sb.tile([128, 2 * S], F16, name=f"zt{t}_{p}", tag="zt", bufs=4)
                    for p in range(2)
                ]
                for k in range(HT):
                    j = 4 + k
                    nc.scalar.activation(
                        out=zt2[k // 2][:, (k % 2) * S : (k % 2) * S + S],
                        in_=z_ps[k // 2][:, (k % 2) * 256 : (k % 2) * 256 + 256],
                        func=AF.Sigmoid, bias=brz_sb[:, j : j + 1], scale=1.0)
                # -- n gate --
                st2 = [
                    sb.tile([128, 2 * S], F16, name=f"st{t}_{p}", tag="stt", bufs=4)
                    for p in range(2)
                ]
                for k in range(HT):
                    nc.vector.scalar_tensor_tensor(
                        out=st2[k // 2][:, (k % 2) * S : (k % 2) * S + S],
                        in0=gh_ps[k // 2][:, (k % 2) * 256 : (k % 2) * 256 + 256],
                        scalar=bhn_sb[:, k : k + 1], in1=r_t[k],
                        op0=ALU.add, op1=ALU.mult)
                nt2, hn2 = [], []
                for p in range(2):
                    u = sb.tile([128, 2 * S], F16, name=f"u{t}_{p}", tag="u", bufs=4)
                    nc.vector.tensor_add(u, st2[p], gxs2[p])
                    nt = sb.tile([128, 2 * S], F16, name=f"nt{t}_{p}", tag="nt", bufs=4)
                    nc.scalar.activation(
                        out=nt, in_=u, func=AF.Tanh, scale=1.0)
                    nt2.append(nt)
                # -- hidden update: hn = (s - n)*z + n --
                for p in range(2):
                    t1 = sb.tile([128, 2 * S], F16, name=f"t1{t}_{p}", tag="t1", bufs=4)
                    nc.vector.tensor_sub(t1, s_cur[:, p * 2 * S : (p + 1) * 2 * S], nt2[p])
                    t2 = sb.tile([128, 2 * S], F16, name=f"t2{t}_{p}", tag="t2", bufs=4)
                    nc.vector.tensor_mul(t2, t1, zt2[p])
                    hh = sb.tile([128, 2 * S], F16, name=f"hn{t}_{p}", tag="hn", bufs=4)
                    nc.vector.tensor_add(hh, t2, nt2[p])
                    hn2.append(hh)
                hn = [hn2[k // 2][:, (k % 2) * S : (k % 2) * S + S] for k in range(HT)]

                # -- residual into LN block buffer --
                if main:
                    if toff == 0:
                        y_blk = [
                            sb.tile([128, BLK * S], F16, name=f"yb{blk}_{k}",
                                    tag=f"yb{k}", bufs=2)
                            for k in range(HT)
                        ]
                        y2 = [
                            sb.tile([128, BLK * S], F16, name=f"y2_{blk}_{k}",
                                    tag=f"y2{k}", bufs=2)
                            for k in range(HT)
                        ]
                    for k in range(HT):
                        ysl = y_blk[k][:, toff * S : (toff + 1) * S]
                        yeng = nc.gpsimd if k % 2 else nc.vector
                        yeng.tensor_add(ysl, hn[k], xsl(xt, k))
                        nc.gpsimd.tensor_mul(
                            y2[k][:, toff * S : (toff + 1) * S], ysl, ysl)
                    if toff == BLK - 1:
                        pend = (blk, y_blk, y2)

                # -- next state (masked), h0 injection at entry to main --
                if t + 1 < T:
                    s_nxt = sb.tile([128, HT * S], F16, name=f"s{t + 1}",
                                    tag="state", bufs=4)
                    for p in range(2):
                        mk2 = msk[:, (t + 1) * 2 * S : (t + 2) * 2 * S]
                        nc.vector.tensor_mul(
                            s_nxt[:, p * 2 * S : (p + 1) * 2 * S], hn2[p], mk2)
                    if t + 1 == R:
                        # chunk-0 columns get the true (masked) h0
                        for k in range(HT):
                            nc.gpsimd.tensor_copy(
                                s_nxt[:, k * S : k * S + S : C],
                                h0m_sb[:, k * NB : (k + 1) * NB])
                    s_cur = s_nxt
                    xt = xt_nxt
                    gx_ps_cur = gx_ps_nxt
                    r_ps = r_nxt

                # -- deferred LN stats chain / apply (emitted at step
                # bottom: in-order engine queues run gate-critical ops first)
                if pend_stats is not None:
                    pblk, pyb, mu_ps, ss_ps = pend_stats
                    FB = BLK * S
                    mu_s = sb.tile([1, FB], F16, name=f"mus{pblk}", tag="mus", bufs=2)
                    ss_s = sb.tile([1, FB], F32, name=f"sss{pblk}", tag="sss", bufs=2)
                    for half in range(2):
                        nc.vector.tensor_scalar_mul(
                            mu_s[:, half * 512 : (half + 1) * 512], mu_ps[half],
                            1.0 / H)
                        nc.vector.tensor_scalar_mul(
                            ss_s[:, half * 512 : (half + 1) * 512], ss_ps[half],
                            1.0 / H)
                    var_s = sb.tile([1, FB], F32, name=f"var{pblk}", tag="vars", bufs=2)
                    nc.vector.scalar_tensor_tensor(
                        out=var_s, in0=mu_s, scalar=-1.0, in1=mu_s,
                        op0=ALU.mult, op1=ALU.mult)
                    nc.vector.tensor_add(var_s, var_s, ss_s)
                    std_s = sb.tile([1, FB], F32, name=f"std{pblk}", tag="stds", bufs=2)
                    nc.scalar.activation(
                        out=std_s, in_=var_s, func=AF.Sqrt, bias=eps_sb, scale=1.0)
                    rst_s = sb.tile([1, FB], F32, name=f"rst{pblk}", tag="rsts", bufs=2)
                    nc.vector.reciprocal_approx_fast(out=rst_s, in_=std_s)
                    rst_h = sb.tile([1, FB], F16, name=f"rsh{pblk}", tag="rsth", bufs=2)
                    nc.vector.tensor_copy(rst_h, rst_s)
                    nc.scalar.dma_start(out=scr[pblk : pblk + 1, 0:1024], in_=mu_s)
                    nc.scalar.dma_start(out=scr[pblk : pblk + 1, 1024:2048], in_=rst_h)
                    mu_bc = sb.tile([128, FB], F16, name=f"mubc{pblk}", tag="mubc", bufs=2)
                    rs_bc = sb.tile([128, FB], F16, name=f"rsbc{pblk}", tag="rsbc", bufs=2)
                    nc.scalar.dma_start(out=mu_bc, in_=_bcast_ap(scr[pblk, 0:1024]))
                    nc.scalar.dma_start(out=rs_bc, in_=_bcast_ap(scr[pblk, 1024:2048]))
                    pend_apply = (pblk, pyb, mu_bc, rs_bc, list(range(HT)))
                    pend_stats = None
                elif pend_apply is not None and (not main or toff == 2):
                    pblk, pyb, mu_bc, rs_bc, ks = pend_apply
                    FB = BLK * S
                    half = ks
                    for k in half:
                        yn = sb.tile([128, FB], F16, name=f"yn{pblk}_{k}", tag="yn", bufs=2)
                        nc.vector.tensor_sub(yn, pyb[k], mu_bc)
                        nc.vector.tensor_mul(yn, yn, rs_bc)
                        if not triv_gb:
                            nc.vector.tensor_scalar(
                                out=yn, in0=yn,
                                scalar1=gam_sb[:, k : k + 1],
                                scalar2=bet_sb[:, k : k + 1],
                                op0=ALU.mult, op1=ALU.add)
                        nc.sync.dma_start(
                            out=out_flat[k][:, pblk * FB : (pblk + 1) * FB], in_=yn)
                    rest = [k for k in ks if k not in half]
                    pend_apply = (pblk, pyb, mu_bc, rs_bc, rest) if rest else None

                # -- final hidden state (chunk C-1 columns) --
                if t == T - 1:
                    hlb = sb.tile([128, HT * NB], F16, name="hlb", tag="hlb", bufs=1)
                    for k in range(HT):
                        base = (k % 2) * S
                        nc.vector.tensor_copy(
                            hlb[:, k * NB : (k + 1) * NB],
                            hn2[k // 2][:, base + C - 1 : base + S : C])
                    for k in range(HT):
                        nc.sync.dma_start(
                            out=hl_d[k, :, :],
                            in_=hlb[:, k * NB : (k + 1) * NB])

            # ---- tail: LN for the last two pending blocks ----
            for tail in range(2):
                if pend is not None:
                    pblk, pyb, py2 = pend
                    FB = BLK * S
                    mu_ps = [
                        ip.tile([1, 512], F32, name=f"mu{pblk}_{h}", tag="ip")
                        for h in range(2)
                    ]
                    ss_ps = [
                        ip.tile([1, 512], F32, name=f"ss{pblk}_{h}", tag="ip")
                        for h in range(2)
                    ]
                    for half in range(2):
                        for k in range(HT):
                            nc.tensor.matmul(
                                mu_ps[half], ones_sb,
                                pyb[k][:, half * 512 : (half + 1) * 512],
                                start=(k == 0), stop=(k == HT - 1))
                        for k in range(HT):
                            nc.tensor.matmul(
                                ss_ps[half], ones_sb,
                                py2[k][:, half * 512 : (half + 1) * 512],
                                start=(k == 0), stop=(k == HT - 1))
                    mu_s = sb.tile([1, FB], F16, name=f"mus{pblk}", tag="mus", bufs=2)
                    ss_s = sb.tile([1, FB], F32, name=f"sss{pblk}", tag="sss", bufs=2)
                    for half in range(2):
                        nc.vector.tensor_scalar_mul(
                            mu_s[:, half * 512 : (half + 1) * 512], mu_ps[half],
                            1.0 / H)
                        nc.vector.tensor_scalar_mul(
                            ss_s[:, half * 512 : (half + 1) * 512], ss_ps[half],
                            1.0 / H)
                    var_s = sb.tile([1, FB], F32, name=f"var{pblk}", tag="vars", bufs=2)
                    nc.vector.scalar_tensor_tensor(
                        out=var_s, in0=mu_s, scalar=-1.0, in1=mu_s,
                        op0=ALU.mult, op1=ALU.mult)
                    nc.vector.tensor_add(var_s, var_s, ss_s)
                    std_s = sb.tile([1, FB], F32, name=f"std{pblk}", tag="stds", bufs=2)
                    nc.scalar.activation(
                        out=std_s, in_=var_s, func=AF.Sqrt, bias=eps_sb, scale=1.0)
                    rst_s = sb.tile([1, FB], F32, name=f"rst{pblk}", tag="rsts", bufs=2)
                    nc.vector.reciprocal_approx_fast(out=rst_s, in_=std_s)
                    rst_h = sb.tile([1, FB], F16, name=f"rsh{pblk}", tag="rsth", bufs=2)
                    nc.vector.tensor_copy(rst_h, rst_s)
                    nc.scalar.dma_start(out=scr[pblk : pblk + 1, 0:1024], in_=mu_s)
                    nc.scalar.dma_start(out=scr[pblk : pblk + 1, 1024:2048], in_=rst_h)
                    mu_bc = sb.tile([128, FB], F16, name=f"mubc{pblk}", tag="mubc", bufs=2)
                    rs_bc = sb.tile([128, FB], F16, name=f"rsbc{pblk}", tag="rsbc", bufs=2)
                    nc.scalar.dma_start(out=mu_bc, in_=_bcast_ap(scr[pblk, 0:1024]))
                    nc.scalar.dma_start(out=rs_bc, in_=_bcast_ap(scr[pblk, 1024:2048]))
                    pend = None
                    pend2 = (pblk, pyb, mu_bc, rs_bc, list(range(HT)))
                if pend_apply is not None:
                    pblk, pyb, mu_bc, rs_bc, ks = pend_apply
                    FB = BLK * S
                    for k in ks:
                        yn = sb.tile([128, FB], F16, name=f"yn{pblk}_{k}", tag="yn", bufs=2)
                        nc.vector.tensor_sub(yn, pyb[k], mu_bc)
                        nc.vector.tensor_mul(yn, yn, rs_bc)
                        if not triv_gb:
                            nc.vector.tensor_scalar(
                                out=yn, in0=yn,
                                scalar1=gam_sb[:, k : k + 1],
                                scalar2=bet_sb[:, k : k + 1],
                                op0=ALU.mult, op1=ALU.add)
                        nc.sync.dma_start(
                            out=out_flat[k][:, pblk * FB : (pblk + 1) * FB], in_=yn)
                    pend_apply = None
                if tail == 0:
                    pend_apply = pend2
    nc.compile()
    return nc


def stage_inputs(input, h, is_initial, W_ih, W_hh, b_ih, b_hh, gamma, beta, R):
    """Host-side sharding/staging. Returns per-core input maps."""
    T = R + KS
    x = np.asarray(input, np.float16)
    h0 = np.asarray(h, np.float32)
    ii = np.asarray(is_initial).reshape(N, L)
    W_ih = np.asarray(W_ih, np.float32)
    W_hh = np.asarray(W_hh, np.float32)
    b_ih = np.asarray(b_ih, np.float32)
    b_hh = np.asarray(b_hh, np.float32)

    mask = (1.0 - ii).astype(np.float16)  # [N, L]

    # l index per (c, t): warm-up reads the R steps before the chunk;
    # chunk 0's warm-up reads l in [KS-R, KS) (discarded garbage).
    l_for = np.empty((C, T), np.int64)
    for c in range(C):
        for t in range(T):
            l = c * KS + (t - R)
            l_for[c, t] = l if l >= 0 else l + KS

    wihT = np.ascontiguousarray(
        W_ih.T.reshape(HT, 128, 3 * H)).astype(np.float16)
    whhT = np.ascontiguousarray(
        W_hh.T.reshape(HT, 128, 3 * H)).astype(np.float16)
    brz = (b_ih + b_hh)[: 2 * H].reshape(8, 128).T.copy()        # [128, 8]
    bhn = b_hh[2 * H :].reshape(HT, 128).T.copy()                # [128, 4]
    binn = b_ih[2 * H :].reshape(HT, 128).T.copy()
    gam = np.asarray(gamma, np.float32).reshape(HT, 128).T.copy()
    bet = np.asarray(beta, np.float32).reshape(HT, 128).T.copy()
    ones = np.ones((128, 1), np.float16)

    in_maps = []
    for core in range(NCORES):
        n0 = core * NB
        xc = x[n0 : n0 + NB]              # [NB, L, H]
        # xs[t][p, k*S + s] = x[n, l_for[c, t], k*128+p], s = n*C + c
        xg = xc[:, l_for, :]              # [NB, C, T, H]
        xs = np.ascontiguousarray(
            xg.transpose(2, 3, 0, 1).reshape(T, HT, 128, S)
            .transpose(0, 2, 1, 3).reshape(T, 128, HT * S))
        mg = mask[n0 : n0 + NB][:, l_for]  # [NB, C, T]
        msf = mg.transpose(2, 0, 1).reshape(T, 1, S)
        ms = np.ascontiguousarray(
            np.broadcast_to(msf, (T, 2, S)).reshape(1, T * 2 * S))
        m0 = mask[n0 : n0 + NB, 0].astype(np.float32)  # [NB]
        h0m = np.ascontiguousarray(
            (h0[n0 : n0 + NB] * m0[:, None]).T.reshape(HT, 128, NB)
            .transpose(1, 0, 2).reshape(128, HT * NB)).astype(np.float16)
        in_maps.append({
            "xs": xs, "ms": ms, "h0m": h0m,
            "wih": wihT, "whh": whhT, "brz": brz, "bhn": bhn, "bin": binn,
            "gam": gam, "bet": bet, "ones": ones,
        })
    return in_maps


def required_warmup(is_initial):
    """Max distance from a chunk boundary back to the latest reset."""
    ii = np.asarray(is_initial).reshape(N, L)
    need = 0
    for c in range(1, C):
        start = c * KS
        sub = ii[:, :start]
        for n in range(N):
            nz = np.nonzero(sub[n])[0]
            gap = start - nz[-1] if len(nz) else start
            need = max(need, gap)
    return need


def unstage_outputs(results):
    out = np.empty((N, L, H), np.float32)
    h_last = np.empty((N, H), np.float32)
    for core in range(NCORES):
        n0 = core * NB
        st = results[core]["out_st"].astype(np.float32)  # [HT, 128, KS, S]
        o = st.reshape(HT, 128, KS, NB, C).transpose(3, 4, 2, 0, 1)
        out[n0 : n0 + NB] = o.reshape(NB, L, H)
        hl = results[core]["hlast"].astype(np.float32)  # [HT, 128, NB]
        h_last[n0 : n0 + NB] = hl.transpose(2, 0, 1).reshape(NB, H)
    h_exp = np.broadcast_to(h_last[:, None, :], (N, L, H)).copy()
    return out, h_exp


_PROGRAM_CACHE = {}


def kernel(input, h, is_initial, W_ih, W_hh, b_ih, b_hh, gamma, beta):
    R = max(required_warmup(is_initial), 1)
    triv = bool(
        np.all(np.asarray(gamma) == 1.0) and np.all(np.asarray(beta) == 0.0))
    key = (R, triv)
    if key not in _PROGRAM_CACHE:
        _PROGRAM_CACHE[key] = build_program(R, triv_gb=triv)
    nc = _PROGRAM_CACHE[key]
    in_maps = stage_inputs(
        input, h, is_initial, W_ih, W_hh, b_ih, b_hh, gamma, beta, R)
    res = run_bass_kernel_spmd(nc, in_maps, list(range(NCORES))).results
    return unstage_outputs(res)


# revision 23
# speedup vs baseline: 1.1578x; 1.1578x over previous
"""Trainium2 Bass kernel for masked-GRU + residual + LayerNorm.

Problem: N=128 sequences of length L=512, hidden H=512.
  gx = x @ W_ih.T + b_ih            (precomputable input projection)
  per step l: hc = h * (1-is_initial[l]); gh = hc @ W_hh.T + b_hh
    r = sig(gx_r+gh_r); z = sig(gx_z+gh_z); n = tanh(gx_n + r*gh_n)
    h = (1-z)*n + z*hc
  out = LayerNorm(seq + x) * gamma + beta;  h_exp = broadcast(h_last)

Strategy:
  * Data parallel: 16 batch rows per core (8 cores).
  * Sequence-chunk parallel: each L=512 sequence is split into C=16
    chunks of 32 steps, made exact by an R-step warm-up (state entering
    a chunk only depends on inputs back to the latest reset; R covers
    the max reset gap, checked at runtime). Chunk 0 injects true h0.
  * fp16 end-to-end: matmul operands, state, gates, outputs (validated
    vs f64 reference: ~1.6e-3 max rel err).
  * Per-step whh matmuls in PSUM; wih prefilled one step ahead (r and
    gx_n groups) to keep the in-order PE queue busy during gate math.
  * LayerNorm mu/ss matmuls + stats + apply are DEFERRED one step so
    they never stall the in-order Tensor queue on the gate chain.
  * All masks preloaded once (partition-broadcast DMA); x loaded with
    one DMA per step; state init via memset.
"""
import sys

sys.path.insert(0, "/opt/trn_rl_repo")

import numpy as np

import concourse.bass as bass
import concourse.tile as tile
from concourse import bacc, mybir
from concourse.bass_utils import run_bass_kernel_spmd

F32 = mybir.dt.float32
F16 = mybir.dt.float16
AF = mybir.ActivationFunctionType
ALU = mybir.AluOpType

N, L, H = 128, 512, 512
NCORES = 8
NB = N // NCORES          # batch rows per core = 16
C = 16                    # chunks per sequence
KS = L // C               # main steps per chunk = 32
S = NB * C                # columns per core = 256
HT = H // 128             # h partition tiles = 4
BLK = 4                   # LN block (main steps)
NBLK = KS // BLK          # 8


def _bcast_ap(row_ap, parts=128):
    """DRAM row AP -> partition-broadcast AP (step 0 over partitions)."""
    return bass.AP(
        tensor=row_ap.tensor,
        offset=row_ap.offset,
        ap=[[0, parts]] + [list(d) for d in row_ap.ap],
    )


def build_program(R=16, triv_gb=False):
    T = R + KS
    nc = bacc.Bacc("TRN2", target_bir_lowering=False)

    xs_d = nc.declare_dram_parameter("xs", [T, 128, HT * S], F16, isOutput=False)
    ms_d = nc.declare_dram_parameter("ms", [1, T * 2 * S], F16, isOutput=False)
    h0m_d = nc.declare_dram_parameter("h0m", [128, HT * NB], F16, isOutput=False)
    wih_d = nc.declare_dram_parameter("wih", [HT, 128, 3 * H], F16, isOutput=False)
    whh_d = nc.declare_dram_parameter("whh", [HT, 128, 3 * H], F16, isOutput=False)
    brz_d = nc.declare_dram_parameter("brz", [128, 8], F32, isOutput=False)
    bhn_d = nc.declare_dram_parameter("bhn", [128, HT], F32, isOutput=False)
    bin_d = nc.declare_dram_parameter("bin", [128, HT], F32, isOutput=False)
    gam_d = nc.declare_dram_parameter("gam", [128, HT], F32, isOutput=False)
    bet_d = nc.declare_dram_parameter("bet", [128, HT], F32, isOutput=False)
    ones_d = nc.declare_dram_parameter("ones", [128, 1], F16, isOutput=False)

    out_d = nc.declare_dram_parameter("out_st", [HT, 128, KS, S], F16, isOutput=True)
    hl_d = nc.declare_dram_parameter("hlast", [HT, 128, NB], F16, isOutput=True)

    scr = nc.dram_tensor("lnscr", [NBLK, 2048], F16)

    with tile.TileContext(nc) as tc:
        with (
            tc.tile_pool(name="const", bufs=1) as cst,
            tc.tile_pool(name="sb", bufs=1) as sb,
            tc.tile_pool(name="rp", bufs=4, space="PSUM") as rp,
            tc.tile_pool(name="ip", bufs=4, space="PSUM") as ip,
        ):
            # ---- constants (wih first: needed by the t=0 prefill) ----
            wih_sb, whh_sb = [], []
            x0 = sb.tile([128, HT * S], F16, name="xt0", tag="xt", bufs=4)
            nc.sync.dma_start(out=x0, in_=xs_d[0, :, :])
            for k in range(HT):
                w1 = cst.tile([128, 3 * H], F16, name=f"wih_sb{k}", tag=f"wih{k}")
                nc.sync.dma_start(out=w1, in_=wih_d[k, :, :])
                wih_sb.append(w1)
            out_flat = [out_d[k, :, :, :].rearrange("p t s -> p (t s)") for k in range(HT)]

            def xsl(xt, k):
                return xt[:, k * S : (k + 1) * S]

            def ssl(st, k):
                return st[:, k * S : (k + 1) * S]

            def load_x(t):
                xt = sb.tile([128, HT * S], F16, name=f"xt{t}", tag="xt", bufs=4)
                nc.sync.dma_start(out=xt, in_=xs_d[t, :, :])
                return xt

            def prefill_gxn(t, xt):
                # complete psum groups for gx_n of step t (wih only),
                # drained straight to SBUF on ACT with b_in folded in
                gx_ps = [
                    ip.tile([128, 512], F32, name=f"gx{t}_{j}", tag="ip")
                    for j in range(2)
                ]
                for k4 in range(4):
                    j = 8 + k4
                    oap = gx_ps[k4 // 2][:, (k4 % 2) * 256 : (k4 % 2) * 256 + 256]
                    for k in range(HT):
                        nc.tensor.matmul(
                            oap, wih_sb[k][:, j * 128 : (j + 1) * 128], xsl(xt, k),
                            start=(k == 0), stop=(k == HT - 1))
                return gx_ps

            def prefill_r(t, xt):
                # open accumulation groups for the r gate of step t
                r_ps = [
                    rp.tile([128, 256], F32, name=f"r{t}_{j}", tag="rp")
                    for j in range(4)
                ]
                for j in range(4):
                    for k in range(HT):
                        nc.tensor.matmul(
                            r_ps[j], wih_sb[k][:, j * 128 : (j + 1) * 128], xsl(xt, k),
                            start=(k == 0), stop=False)
                return r_ps

            xt = x0
            gx_ps_cur = prefill_gxn(0, xt)
            r_ps = prefill_r(0, xt)

            # ---- remaining constants (loaded while the prefill runs) ----
            for k in range(HT):
                w2 = cst.tile([128, 3 * H], F16, name=f"whh_sb{k}", tag=f"whh{k}")
                nc.sync.dma_start(out=w2, in_=whh_d[k, :, :])
                whh_sb.append(w2)
            h0m_sb = cst.tile([128, HT * NB], F16, name="h0m_sb", tag="h0m")
            nc.sync.dma_start(out=h0m_sb, in_=h0m_d[:, :])
            brz_sb = cst.tile([128, 8], F32, name="brz_sb", tag="brz")
            nc.sync.dma_start(out=brz_sb, in_=brz_d[:, :])
            bhn_sb = cst.tile([128, HT], F32, name="bhn_sb", tag="bhn")
            nc.sync.dma_start(out=bhn_sb, in_=bhn_d[:, :])
            bin_sb = cst.tile([128, HT], F32, name="bin_sb", tag="bin")
            nc.sync.dma_start(out=bin_sb, in_=bin_d[:, :])
            gam_sb = bet_sb = None
            if not triv_gb:
                gam_sb = cst.tile([128, HT], F32, name="gam_sb", tag="gam")
                nc.sync.dma_start(out=gam_sb, in_=gam_d[:, :])
                bet_sb = cst.tile([128, HT], F32, name="bet_sb", tag="bet")
                nc.sync.dma_start(out=bet_sb, in_=bet_d[:, :])
            ones_sb = cst.tile([128, 1], F16, name="ones_sb", tag="ones")
            nc.sync.dma_start(out=ones_sb, in_=ones_d[:, :])
            eps_sb = cst.tile([1, 1], F32, name="eps_sb", tag="eps")
            nc.vector.memset(eps_sb, 1e-5)
            msk = cst.tile([128, T * 2 * S], F16, name="msk", tag="msk")
            MH = 8 * 2 * S
            nc.sync.dma_start(out=msk[:, 0:MH], in_=_bcast_ap(ms_d[0, 0:MH]))
            nc.sync.dma_start(
                out=msk[:, MH : T * 2 * S], in_=_bcast_ap(ms_d[0, MH : T * 2 * S]))

            # ---- initial (zero) state: one [128, HT*S] tile ----
            s_cur = sb.tile([128, HT * S], F16, name="s_init", tag="state", bufs=4)
            nc.vector.memset(s_cur, 0.0)

            y_blk = None
            y2 = None
            pend = None          # (blk, y_blk, y2) finished, LN deferred
            pend_stats = None    # (blk, y_blk, mu_ps, ss_ps) stats deferred
            pend_apply = None    # (blk, y_blk, mu_bc, rs_bc) apply deferred
            for t in range(T):
                main = t >= R
                toff = (t - R) % BLK
                blk = (t - R) // BLK

                # -- gx_n drain (to SBUF, b_in folded): frees its psum
                # slots immediately and fills the engine-idle step start
                gxs2 = [
                    sb.tile([128, 2 * S], F16, name=f"gxs{t}_{p}", tag="gxs", bufs=4)
                    for p in range(2)
                ]
                for k in range(HT):
                    psl = gx_ps_cur[k // 2][:, (k % 2) * 256 : (k % 2) * 256 + 256]
                    nc.scalar.activation(
                        out=gxs2[k // 2][:, (k % 2) * S : (k % 2) * S + S],
                        in_=psl, func=AF.Identity,
                        bias=bin_sb[:, k : k + 1], scale=1.0)

                # ---- deferred LN stats for the block finished last step:
                # mu/ss matmuls go FIRST on the in-order PE queue (their
                # inputs are old => no stall), then the stats chain.
                if pend is not None and (not main or toff == 1):
                    # only the mu/ss matmuls here: they fill the PE lull at
                    # the step start; the stats chain is emitted at the
                    # step BOTTOM so it queues behind the critical gate ops
                    pblk, pyb, py2 = pend
                    mu_ps = [
                        ip.tile([1, 512], F32, name=f"mu{pblk}_{h}", tag="ip")
                        for h in range(2)
                    ]
                    ss_ps = [
                        ip.tile([1, 512], F32, name=f"ss{pblk}_{h}", tag="ip")
                        for h in range(2)
                    ]
                    for half in range(2):
                        for k in range(HT):
                            nc.tensor.matmul(
                                mu_ps[half], ones_sb,
                                pyb[k][:, half * 512 : (half + 1) * 512],
                                start=(k == 0), stop=(k == HT - 1))
                        for k in range(HT):
                            nc.tensor.matmul(
                                ss_ps[half], ones_sb,
                                py2[k][:, half * 512 : (half + 1) * 512],
                                start=(k == 0), stop=(k == HT - 1))
                    pend_stats = (pblk, pyb, mu_ps, ss_ps)
                    pend = None

                if t + 1 < T:
                    xt_nxt = load_x(t + 1)

                # -- close r groups with the recurrent part --
                # j-outer: each r group closes after its 4 matmuls, so
                # the r sigmoids start ~1.3us earlier and overlap the gh_n
                # matmul phase instead of serializing after it
                for j in range(4):
                    for k in range(HT):
                        nc.tensor.matmul(
                            r_ps[j], whh_sb[k][:, j * 128 : (j + 1) * 128],
                            ssl(s_cur, k), start=False, stop=(k == HT - 1))
                # -- gh_n (whh only, complete groups) --
                gh_ps = [
                    ip.tile([128, 512], F32, name=f"gh{t}_{j}", tag="ip")
                    for j in range(2)
                ]
                for k4 in range(4):
                    j = 8 + k4
                    oap = gh_ps[k4 // 2][:, (k4 % 2) * 256 : (k4 % 2) * 256 + 256]
                    for k in range(HT):
                        nc.tensor.matmul(
                            oap, whh_sb[k][:, j * 128 : (j + 1) * 128], ssl(s_cur, k),
                            start=(k == 0), stop=(k == HT - 1))
                # -- z gate (whh + wih complete groups, in-step) --
                z_ps = [
                    ip.tile([128, 512], F32, name=f"z{t}_{j}", tag="ip")
                    for j in range(2)
                ]
                for j4 in range(4):
                    j = 4 + j4
                    oap = z_ps[j4 // 2][:, (j4 % 2) * 256 : (j4 % 2) * 256 + 256]
                    for k in range(HT):
                        nc.tensor.matmul(
                            oap, whh_sb[k][:, j * 128 : (j + 1) * 128], ssl(s_cur, k),
                            start=(k == 0), stop=False)
                    for k in range(HT):
                        nc.tensor.matmul(
                            oap, wih_sb[k][:, j * 128 : (j + 1) * 128], xsl(xt, k),
                            start=False, stop=(k == HT - 1))

                # -- prefill next step (PE stays busy during gate math) --
                if t + 1 < T:
                    gx_ps_nxt = prefill_gxn(t + 1, xt_nxt)
                    r_nxt = prefill_r(t + 1, xt_nxt)

                # -- sigmoids straight from PSUM (bias = b_ih + b_hh) --
                r_t, z_t = [], []
                for k in range(HT):
                    rt = sb.tile([128, S], F16, name=f"rt{t}_{k}", tag="rt", bufs=4)
                    nc.scalar.activation(
                        out=rt, in_=r_ps[k],
                        func=AF.Sigmoid, bias=brz_sb[:, k : k + 1], scale=1.0)
                    r_t.append(rt)
                zt2 = [
                    sb.tile([128, 2 * S], F16, name=f"zt{t}_{p}", tag="zt", bufs=4)
                    for p in range(2)
                ]
                for k in range(HT):
                    j = 4 + k
                    nc.scalar.activation(
                        out=zt2[k // 2][:, (k % 2) * S : (k % 2) * S + S],
                        in_=z_ps[k // 2][:, (k % 2) * 256 : (k % 2) * 256 + 256],
                        func=AF.Sigmoid, bias=brz_sb[:, j : j + 1], scale=1.0)
                # -- n gate --
                st2 = [
                    sb.tile([128, 2 * S], F16, name=f"st{t}_{p}", tag="stt", bufs=4)
                    for p in range(2)
                ]
                for k in range(HT):
                    nc.vector.scalar_tensor_tensor(
                        out=st2[k // 2][:, (k % 2) * S : (k % 2) * S + S],
                        in0=gh_ps[k // 2][:, (k % 2) * 256 : (k % 2) * 256 + 256],
                        scalar=bhn_sb[:, k : k + 1], in1=r_t[k],
                        op0=ALU.add, op1=ALU.mult)
                nt2, hn2 = [], []
                for p in range(2):
                    u = sb.tile([128, 2 * S], F16, name=f"u{t}_{p}", tag="u", bufs=4)
                    nc.vector.tensor_add(u, st2[p], gxs2[p])
                    nt = sb.tile([128, 2 * S], F16, name=f"nt{t}_{p}", tag="nt", bufs=4)
                    nc.scalar.activation(
                        out=nt, in_=u, func=AF.Tanh, scale=1.0)
                    nt2.append(nt)
                # -- hidden update: hn = (s - n)*z + n --
                for p in range(2):
                    t1 = sb.tile([128, 2 * S], F16, name=f"t1{t}_{p}", tag="t1", bufs=4)
                    nc.vector.tensor_sub(t1, s_cur[:, p * 2 * S : (p + 1) * 2 * S], nt2[p])
                    t2 = sb.tile([128, 2 * S], F16, name=f"t2{t}_{p}", tag="t2", bufs=4)
                    nc.vector.tensor_mul(t2, t1, zt2[p])
                    hh = sb.tile([128, 2 * S], F16, name=f"hn{t}_{p}", tag="hn", bufs=4)
                    nc.vector.tensor_add(hh, t2, nt2[p])
                    hn2.append(hh)
                hn = [hn2[k // 2][:, (k % 2) * S : (k % 2) * S + S] for k in range(HT)]

                # -- residual into LN block buffer --
                if main:
                    if toff == 0:
                        y_blk = [
                            sb.tile([128, BLK * S], F16, name=f"yb{blk}_{k}",
                                    tag=f"yb{k}", bufs=2)
                            for k in range(HT)
                        ]
                        y2 = [
                            sb.tile([128, BLK * S], F16, name=f"y2_{blk}_{k}",
                                    tag=f"y2{k}", bufs=2)
                            for k in range(HT)
                        ]
                    for k in range(HT):
                        ysl = y_blk[k][:, toff * S : (toff + 1) * S]
                        yeng = nc.gpsimd if k % 2 else nc.vector
                        yeng.tensor_add(ysl, hn[k], xsl(xt, k))
                        nc.gpsimd.tensor_mul(
                            y2[k][:, toff * S : (toff + 1) * S], ysl, ysl)
                    if toff == BLK - 1:
                        pend = (blk, y_blk, y2)

                # -- next state (masked), h0 injection at entry to main --
                if t + 1 < T:
                    s_nxt = sb.tile([128, HT * S], F16, name=f"s{t + 1}",
                                    tag="state", bufs=4)
                    for p in range(2):
                        mk2 = msk[:, (t + 1) * 2 * S : (t + 2) * 2 * S]
                        nc.vector.tensor_mul(
                            s_nxt[:, p * 2 * S : (p + 1) * 2 * S], hn2[p], mk2)
                    if t + 1 == R:
                        # chunk-0 columns get the true (masked) h0
                        for k in range(HT):
                            nc.gpsimd.tensor_copy(
                                s_nxt[:, k * S : k * S + S : C],
                                h0m_sb[:, k * NB : (k + 1) * NB])
                    s_cur = s_nxt
                    xt = xt_nxt
                    gx_ps_cur = gx_ps_nxt
                    r_ps = r_nxt

                # -- deferred LN stats chain / apply (emitted at step
                # bottom: in-order engine queues run gate-critical ops first)
                if pend_stats is not None:
                    pblk, pyb, mu_ps, ss_ps = pend_stats
                    FB = BLK * S
                    mu_s = sb.tile([1, FB], F16, name=f"mus{pblk}", tag="mus", bufs=2)
                    ss_s = sb.tile([1, FB], F32, name=f"sss{pblk}", tag="sss", bufs=2)
                    for half in range(2):
                        nc.vector.tensor_scalar_mul(
                            mu_s[:, half * 512 : (half + 1) * 512], mu_ps[half],
                            1.0 / H)
                        nc.vector.tensor_scalar_mul(
                            ss_s[:, half * 512 : (half + 1) * 512], ss_ps[half],
                            1.0 / H)
                    var_s = sb.tile([1, FB], F32, name=f"var{pblk}", tag="vars", bufs=2)
                    nc.vector.scalar_tensor_tensor(
                        out=var_s, in0=mu_s, scalar=-1.0, in1=mu_s,
                        op0=ALU.mult, op1=ALU.mult)
                    nc.vector.tensor_add(var_s, var_s, ss_s)
                    std_s = sb.tile([1, FB], F32, name=f"std{pblk}", tag="stds", bufs=2)
                    nc.scalar.activation(
                        out=std_s, in_=var_s, func=AF.Sqrt, bias=eps_sb, scale=1.0)
                    rst_s = sb.tile([1, FB], F32, name=f"rst{pblk}", tag="rsts", bufs=2)
                    nc.vector.reciprocal_approx_fast(out=rst_s, in_=std_s)
                    rst_h = sb.tile([1, FB], F16, name=f"rsh{pblk}", tag="rsth", bufs=2)
                    nc.vector.tensor_copy(rst_h, rst_s)
                    nc.scalar.dma_start(out=scr[pblk : pblk + 1, 0:1024], in_=mu_s)
                    nc.scalar.dma_start(out=scr[pblk : pblk + 1, 1024:2048], in_=rst_h)
                    mu_bc = sb.tile([128, FB], F16, name=f"mubc{pblk}", tag="mubc", bufs=2)
                    rs_bc = sb.tile([128, FB], F16, name=f"rsbc{pblk}", tag="rsbc", bufs=2)
                    nc.scalar.dma_start(out=mu_bc, in_=_bcast_ap(scr[pblk, 0:1024]))
                    nc.scalar.dma_start(out=rs_bc, in_=_bcast_ap(scr[pblk, 1024:2048]))
                    pend_apply = (pblk, pyb, mu_bc, rs_bc, list(range(HT)))
                    pend_stats = None
                elif pend_apply is not None and (not main or toff == 2):
                    pblk, pyb, mu_bc, rs_bc, ks = pend_apply
                    FB = BLK * S
                    half = ks
                    for k in half:
                        yn = sb.tile([128, FB], F16, name=f"yn{pblk}_{k}", tag="yn", bufs=2)
                        nc.vector.tensor_sub(yn, pyb[k], mu_bc)
                        nc.vector.tensor_mul(yn, yn, rs_bc)
                        if not triv_gb:
                            nc.vector.tensor_scalar(
                                out=yn, in0=yn,
                                scalar1=gam_sb[:, k : k + 1],
                                scalar2=bet_sb[:, k : k + 1],
                                op0=ALU.mult, op1=ALU.add)
                        nc.sync.dma_start(
                            out=out_flat[k][:, pblk * FB : (pblk + 1) * FB], in_=yn)
                    rest = [k for k in ks if k not in half]
                    pend_apply = (pblk, pyb, mu_bc, rs_bc, rest) if rest else None

                # -- final hidden state (chunk C-1 columns) --
                if t == T - 1:
                    hlb = sb.tile([128, HT * NB], F16, name="hlb", tag="hlb", bufs=1)
                    for k in range(HT):
                        base = (k % 2) * S
                        nc.vector.tensor_copy(
                            hlb[:, k * NB : (k + 1) * NB],
                            hn2[k // 2][:, base + C - 1 : base + S : C])
                    for k in range(HT):
                        nc.sync.dma_start(
                            out=hl_d[k, :, :],
                            in_=hlb[:, k * NB : (k + 1) * NB])

            # ---- tail: LN for the last two pending blocks ----
            for tail in range(2):
                if pend is not None:
                    pblk, pyb, py2 = pend
                    FB = BLK * S
                    mu_ps = [
                        ip.tile([1, 512], F32, name=f"mu{pblk}_{h}", tag="ip")
                        for h in range(2)
                    ]
                    ss_ps = [
                        ip.tile([1, 512], F32, name=f"ss{pblk}_{h}", tag="ip")
                        for h in range(2)
                    ]
                    for half in range(2):
                        for k in range(HT):
                            nc.tensor.matmul(
                                mu_ps[half], ones_sb,
                                pyb[k][:, half * 512 : (half + 1) * 512],
                                start=(k == 0), stop=(k == HT - 1))
                        for k in range(HT):
                            nc.tensor.matmul(
                                ss_ps[half], ones_sb,
                                py2[k][:, half * 512 : (half + 1) * 512],
                                start=(k == 0), stop=(k == HT - 1))
                    mu_s = sb.tile([1, FB], F16, name=f"mus{pblk}", tag="mus", bufs=2)
                    ss_s = sb.tile([1, FB], F32, name=f"sss{pblk}", tag="sss", bufs=2)
                    for half in range(2):
                        nc.vector.tensor_scalar_mul(
                            mu_s[:, half * 512 : (half + 1) * 512], mu_ps[half],
                            1.0 / H)
                        nc.vector.tensor_scalar_mul(
                            ss_s[:, half * 512 : (half + 1) * 512], ss_ps[half],
                            1.0 / H)
                    var_s = sb.tile([1, FB], F32, name=f"var{pblk}", tag="vars", bufs=2)
                    nc.vector.scalar_tensor_tensor(
                        out=var_s, in0=mu_s, scalar=-1.0, in1=mu_s,
                        op0=ALU.mult, op1=ALU.mult)
                    nc.vector.tensor_add(var_s, var_s, ss_s)
                    std_s = sb.tile([1, FB], F32, name=f"std{pblk}", tag="stds", bufs=2)
                    nc.scalar.activation(
                        out=std_s, in_=var_s, func=AF.Sqrt, bias=eps_sb, scale=1.0)
                    rst_s = sb.tile([1, FB], F32, name=f"rst{pblk}", tag="rsts", bufs=2)
                    nc.vector.reciprocal_approx_fast(out=rst_s, in_=std_s)
                    rst_h = sb.tile([1, FB], F16, name=f"rsh{pblk}", tag="rsth", bufs=2)
                    nc.vector.tensor_copy(rst_h, rst_s)
                    nc.scalar.dma_start(out=scr[pblk : pblk + 1, 0:1024], in_=mu_s)
                    nc.scalar.dma_start(out=scr[pblk : pblk + 1, 1024:2048], in_=rst_h)
                    mu_bc = sb.tile([128, FB], F16, name=f"mubc{pblk}", tag="mubc", bufs=2)
                    rs_bc = sb.tile([128, FB], F16, name=f"rsbc{pblk}", tag="rsbc", bufs=2)
                    nc.scalar.dma_start(out=mu_bc, in_=_bcast_ap(scr[pblk, 0:1024]))
                    nc.scalar.dma_start(out=rs_bc, in_=_bcast_ap(scr[pblk, 1024:2048]))
                    pend = None
                    pend2 = (pblk, pyb, mu_bc, rs_bc, list(range(HT)))
                if pend_apply is not None:
                    pblk, pyb, mu_bc, rs_bc, ks = pend_apply
                    FB = BLK * S
                    for k in ks:
                        yn = sb.tile([128, FB], F16, name=f"yn{pblk}_{k}", tag="yn", bufs=2)
                        nc.vector.tensor_sub(yn, pyb[k], mu_bc)
                        nc.vector.tensor_mul(yn, yn, rs_bc)
                        if not triv_gb:
                            nc.vector.tensor_scalar(
                                out=yn, in0=yn,
                                scalar1=gam_sb[:, k : k + 1],
                                scalar2=bet_sb[:, k : k + 1],
                                op0=ALU.mult, op1=ALU.add)
                        nc.sync.dma_start(
                            out=out_flat[k][:, pblk * FB : (pblk + 1) * FB], in_=yn)
                    pend_apply = None
                if tail == 0:
                    pend_apply = pend2
    nc.compile()
    return nc


def stage_inputs(input, h, is_initial, W_ih, W_hh, b_ih, b_hh, gamma, beta, R):
    """Host-side sharding/staging. Returns per-core input maps."""
    T = R + KS
    x = np.asarray(input, np.float16)
    h0 = np.asarray(h, np.float32)
    ii = np.asarray(is_initial).reshape(N, L)
    W_ih = np.asarray(W_ih, np.float32)
    W_hh = np.asarray(W_hh, np.float32)
    b_ih = np.asarray(b_ih, np.float32)
    b_hh = np.asarray(b_hh, np.float32)

    mask = (1.0 - ii).astype(np.float16)  # [N, L]

    # l index per (c, t): warm-up reads the R steps before the chunk;
    # chunk 0's warm-up reads l in [KS-R, KS) (discarded garbage).
    l_for = np.empty((C, T), np.int64)
    for c in range(C):
        for t in range(T):
            l = c * KS + (t - R)
            l_for[c, t] = l if l >= 0 else l + KS

    wihT = np.ascontiguousarray(
        W_ih.T.reshape(HT, 128, 3 * H)).astype(np.float16)
    whhT = np.ascontiguousarray(
        W_hh.T.reshape(HT, 128, 3 * H)).astype(np.float16)
    brz = (b_ih + b_hh)[: 2 * H].reshape(8, 128).T.copy()        # [128, 8]
    bhn = b_hh[2 * H :].reshape(HT, 128).T.copy()                # [128, 4]
    binn = b_ih[2 * H :].reshape(HT, 128).T.copy()
    gam = np.asarray(gamma, np.float32).reshape(HT, 128).T.copy()
    bet = np.asarray(beta, np.float32).reshape(HT, 128).T.copy()
    ones = np.ones((128, 1), np.float16)

    in_maps = []
    for core in range(NCORES):
        n0 = core * NB
        xc = x[n0 : n0 + NB]              # [NB, L, H]
        # xs[t][p, k*S + s] = x[n, l_for[c, t], k*128+p], s = n*C + c
        xg = xc[:, l_for, :]              # [NB, C, T, H]
        xs = np.ascontiguousarray(
            xg.transpose(2, 3, 0, 1).reshape(T, HT, 128, S)
            .transpose(0, 2, 1, 3).reshape(T, 128, HT * S))
        mg = mask[n0 : n0 + NB][:, l_for]  # [NB, C, T]
        msf = mg.transpose(2, 0, 1).reshape(T, 1, S)
        ms = np.ascontiguousarray(
            np.broadcast_to(msf, (T, 2, S)).reshape(1, T * 2 * S))
        m0 = mask[n0 : n0 + NB, 0].astype(np.float32)  # [NB]
        h0m = np.ascontiguousarray(
            (h0[n0 : n0 + NB] * m0[:, None]).T.reshape(HT, 128, NB)
            .transpose(1, 0, 2).reshape(128, HT * NB)).astype(np.float16)
        in_maps.append({
            "xs": xs, "ms": ms, "h0m": h0m,
            "wih": wihT, "whh": whhT, "brz": brz, "bhn": bhn, "bin": binn,
            "gam": gam, "bet": bet, "ones": ones,
        })
    return in_maps


def required_warmup(is_initial):
    """Max distance from a chunk boundary back to the latest reset."""
    ii = np.asarray(is_initial).reshape(N, L)
    need = 0
    for c in range(1, C):
        start = c * KS
        sub = ii[:, :start]
        for n in range(N):
            nz = np.nonzero(sub[n])[0]
            gap = start - nz[-1] if len(nz) else start
            need = max(need, gap)
    return need


def unstage_outputs(results):
    out = np.empty((N, L, H), np.float32)
    h_last = np.empty((N, H), np.float32)
    for core in range(NCORES):
        n0 = core * NB
        st = results[core]["out_st"].astype(np.float32)  # [HT, 128, KS, S]
        o = st.reshape(HT, 128, KS, NB, C).transpose(3, 4, 2, 0, 1)
        out[n0 : n0 + NB] = o.reshape(NB, L, H)
        hl = results[core]["hlast"].astype(np.float32)  # [HT, 128, NB]
        h_last[n0 : n0 + NB] = hl.transpose(2, 0, 1).reshape(NB, H)
    h_exp = np.broadcast_to(h_last[:, None, :], (N, L, H)).copy()
    return out, h_exp


_PROGRAM_CACHE = {}


def kernel(input, h, is_initial, W_ih, W_hh, b_ih, b_hh, gamma, beta):
    R = max(required_warmup(is_initial), 1)
    triv = bool(
        np.all(np.asarray(gamma) == 1.0) and np.all(np.asarray(beta) == 0.0))
    key = (R, triv)
    if key not in _PROGRAM_CACHE:
        _PROGRAM_CACHE[key] = build_program(R, triv_gb=triv)
    nc = _PROGRAM_CACHE[key]
    in_maps = stage_inputs(
        input, h, is_initial, W_ih, W_hh, b_ih, b_hh, gamma, beta, R)
    res = run_bass_kernel_spmd(nc, in_maps, list(range(NCORES))).results
    return unstage_outputs(res)


# revision 24
# speedup vs baseline: 1.1823x; 1.0212x over previous
"""Trainium2 Bass kernel for masked-GRU + residual + LayerNorm.

Problem: N=128 sequences of length L=512, hidden H=512.
  gx = x @ W_ih.T + b_ih            (precomputable input projection)
  per step l: hc = h * (1-is_initial[l]); gh = hc @ W_hh.T + b_hh
    r = sig(gx_r+gh_r); z = sig(gx_z+gh_z); n = tanh(gx_n + r*gh_n)
    h = (1-z)*n + z*hc
  out = LayerNorm(seq + x) * gamma + beta;  h_exp = broadcast(h_last)

Strategy:
  * Data parallel: 16 batch rows per core (8 cores).
  * Sequence-chunk parallel: each L=512 sequence is split into C=16
    chunks of 32 steps, made exact by an R-step warm-up (state entering
    a chunk only depends on inputs back to the latest reset; R covers
    the max reset gap, checked at runtime). Chunk 0 injects true h0.
  * fp16 end-to-end: matmul operands, state, gates, outputs (validated
    vs f64 reference: ~1.6e-3 max rel err).
  * Per-step whh matmuls in PSUM; wih prefilled one step ahead (r and
    gx_n groups) to keep the in-order PE queue busy during gate math.
  * LayerNorm mu/ss matmuls + stats + apply are DEFERRED one step so
    they never stall the in-order Tensor queue on the gate chain.
  * All masks preloaded once (partition-broadcast DMA); x loaded with
    one DMA per step; state init via memset.
"""
import sys

sys.path.insert(0, "/opt/trn_rl_repo")

import numpy as np

import concourse.bass as bass
import concourse.tile as tile
from concourse import bacc, mybir
from concourse.bass_utils import run_bass_kernel_spmd

F32 = mybir.dt.float32
F16 = mybir.dt.float16
AF = mybir.ActivationFunctionType
ALU = mybir.AluOpType

N, L, H = 128, 512, 512
NCORES = 8
NB = N // NCORES          # batch rows per core = 16
C = 16                    # chunks per sequence
KS = L // C               # main steps per chunk = 32
S = NB * C                # columns per core = 256
HT = H // 128             # h partition tiles = 4
BLK = 4                   # LN block (main steps)
NBLK = KS // BLK          # 8


def _bcast_ap(row_ap, parts=128):
    """DRAM row AP -> partition-broadcast AP (step 0 over partitions)."""
    return bass.AP(
        tensor=row_ap.tensor,
        offset=row_ap.offset,
        ap=[[0, parts]] + [list(d) for d in row_ap.ap],
    )


def build_program(R=16, triv_gb=False):
    T = R + KS
    nc = bacc.Bacc("TRN2", target_bir_lowering=False)

    xs_d = nc.declare_dram_parameter("xs", [T, 128, HT * S], F16, isOutput=False)
    ms_d = nc.declare_dram_parameter("ms", [1, T * 2 * S], F16, isOutput=False)
    h0m_d = nc.declare_dram_parameter("h0m", [128, HT * NB], F16, isOutput=False)
    wih_d = nc.declare_dram_parameter("wih", [HT, 128, 3 * H], F16, isOutput=False)
    whh_d = nc.declare_dram_parameter("whh", [HT, 128, 3 * H], F16, isOutput=False)
    brz_d = nc.declare_dram_parameter("brz", [128, 8], F32, isOutput=False)
    bhn_d = nc.declare_dram_parameter("bhn", [128, HT], F32, isOutput=False)
    bin_d = nc.declare_dram_parameter("bin", [128, HT], F32, isOutput=False)
    gam_d = nc.declare_dram_parameter("gam", [128, HT], F32, isOutput=False)
    bet_d = nc.declare_dram_parameter("bet", [128, HT], F32, isOutput=False)
    ones_d = nc.declare_dram_parameter("ones", [128, 1], F16, isOutput=False)

    out_d = nc.declare_dram_parameter("out_st", [HT, 128, KS, S], F16, isOutput=True)
    hl_d = nc.declare_dram_parameter("hlast", [HT, 128, NB], F16, isOutput=True)

    scr = nc.dram_tensor("lnscr", [NBLK, 2048], F16)

    with tile.TileContext(nc) as tc:
        with (
            tc.tile_pool(name="const", bufs=1) as cst,
            tc.tile_pool(name="sb", bufs=1) as sb,
            tc.tile_pool(name="rp", bufs=4, space="PSUM") as rp,
            tc.tile_pool(name="ip", bufs=4, space="PSUM") as ip,
        ):
            # ---- constants (wih first: needed by the t=0 prefill) ----
            wih_sb, whh_sb = [], []
            x0 = sb.tile([128, HT * S], F16, name="xt0", tag="xt", bufs=4)
            nc.sync.dma_start(out=x0, in_=xs_d[0, :, :])
            for k in range(HT):
                w1 = cst.tile([128, 3 * H], F16, name=f"wih_sb{k}", tag=f"wih{k}")
                nc.sync.dma_start(out=w1, in_=wih_d[k, :, :])
                wih_sb.append(w1)
            out_flat = [out_d[k, :, :, :].rearrange("p t s -> p (t s)") for k in range(HT)]

            def xsl(xt, k):
                return xt[:, k * S : (k + 1) * S]

            def ssl(st, k):
                return st[:, k * S : (k + 1) * S]

            def load_x(t):
                xt = sb.tile([128, HT * S], F16, name=f"xt{t}", tag="xt", bufs=4)
                nc.sync.dma_start(out=xt, in_=xs_d[t, :, :])
                return xt

            def prefill_gxn(t, xt):
                # complete psum groups for gx_n of step t (wih only),
                # drained straight to SBUF on ACT with b_in folded in
                gx_ps = [
                    ip.tile([128, 512], F32, name=f"gx{t}_{j}", tag="ip")
                    for j in range(2)
                ]
                for k4 in range(4):
                    j = 8 + k4
                    oap = gx_ps[k4 // 2][:, (k4 % 2) * 256 : (k4 % 2) * 256 + 256]
                    for k in range(HT):
                        nc.tensor.matmul(
                            oap, wih_sb[k][:, j * 128 : (j + 1) * 128], xsl(xt, k),
                            start=(k == 0), stop=(k == HT - 1))
                return gx_ps

            def prefill_r(t, xt):
                # open accumulation groups for the r gate of step t
                r_ps = [
                    rp.tile([128, 256], F32, name=f"r{t}_{j}", tag="rp")
                    for j in range(4)
                ]
                for j in range(4):
                    for k in range(HT):
                        nc.tensor.matmul(
                            r_ps[j], wih_sb[k][:, j * 128 : (j + 1) * 128], xsl(xt, k),
                            start=(k == 0), stop=False)
                return r_ps

            xt = x0
            gx_ps_cur = prefill_gxn(0, xt)
            r_ps = prefill_r(0, xt)

            # ---- remaining constants (loaded while the prefill runs) ----
            for k in range(HT):
                w2 = cst.tile([128, 3 * H], F16, name=f"whh_sb{k}", tag=f"whh{k}")
                nc.sync.dma_start(out=w2, in_=whh_d[k, :, :])
                whh_sb.append(w2)
            h0m_sb = cst.tile([128, HT * NB], F16, name="h0m_sb", tag="h0m")
            nc.sync.dma_start(out=h0m_sb, in_=h0m_d[:, :])
            brz_sb = cst.tile([128, 8], F32, name="brz_sb", tag="brz")
            nc.sync.dma_start(out=brz_sb, in_=brz_d[:, :])
            bhn_sb = cst.tile([128, HT], F32, name="bhn_sb", tag="bhn")
            nc.sync.dma_start(out=bhn_sb, in_=bhn_d[:, :])
            bin_sb = cst.tile([128, HT], F32, name="bin_sb", tag="bin")
            nc.sync.dma_start(out=bin_sb, in_=bin_d[:, :])
            gam_sb = bet_sb = None
            if not triv_gb:
                gam_sb = cst.tile([128, HT], F32, name="gam_sb", tag="gam")
                nc.sync.dma_start(out=gam_sb, in_=gam_d[:, :])
                bet_sb = cst.tile([128, HT], F32, name="bet_sb", tag="bet")
                nc.sync.dma_start(out=bet_sb, in_=bet_d[:, :])
            ones_sb = cst.tile([128, 1], F16, name="ones_sb", tag="ones")
            nc.sync.dma_start(out=ones_sb, in_=ones_d[:, :])
            eps_sb = cst.tile([1, 1], F32, name="eps_sb", tag="eps")
            nc.vector.memset(eps_sb, 1e-5)
            msk = cst.tile([128, T * 2 * S], F16, name="msk", tag="msk")
            MH = 8 * 2 * S
            nc.sync.dma_start(out=msk[:, 0:MH], in_=_bcast_ap(ms_d[0, 0:MH]))
            nc.sync.dma_start(
                out=msk[:, MH : T * 2 * S], in_=_bcast_ap(ms_d[0, MH : T * 2 * S]))

            # ---- initial (zero) state: one [128, HT*S] tile ----
            s_cur = sb.tile([128, HT * S], F16, name="s_init", tag="state", bufs=4)
            nc.vector.memset(s_cur, 0.0)

            y_blk = None
            y2 = None
            pend = None          # (blk, y_blk, y2) finished, LN deferred
            pend_stats = None    # (blk, y_blk, mu_ps, ss_ps) stats deferred
            pend_apply = None    # (blk, y_blk, mu_bc, rs_bc) apply deferred
            for t in range(T):
                main = t >= R
                toff = (t - R) % BLK
                blk = (t - R) // BLK

                # -- gx_n drain (to SBUF, b_in folded): frees its psum
                # slots immediately and fills the engine-idle step start
                gxs2 = [
                    sb.tile([128, 2 * S], F16, name=f"gxs{t}_{p}", tag="gxs", bufs=4)
                    for p in range(2)
                ]
                for k in range(HT):
                    psl = gx_ps_cur[k // 2][:, (k % 2) * 256 : (k % 2) * 256 + 256]
                    nc.scalar.activation(
                        out=gxs2[k // 2][:, (k % 2) * S : (k % 2) * S + S],
                        in_=psl, func=AF.Identity,
                        bias=bin_sb[:, k : k + 1], scale=1.0)

                # ---- deferred LN stats for the block finished last step:
                # mu/ss matmuls go FIRST on the in-order PE queue (their
                # inputs are old => no stall), then the stats chain.
                if pend is not None and (not main or toff == 1):
                    # only the mu/ss matmuls here: they fill the PE lull at
                    # the step start; the stats chain is emitted at the
                    # step BOTTOM so it queues behind the critical gate ops
                    pblk, pyb, py2 = pend
                    mu_ps = [
                        ip.tile([1, 512], F32, name=f"mu{pblk}_{h}", tag="ip")
                        for h in range(2)
                    ]
                    ss_ps = [
                        ip.tile([1, 512], F32, name=f"ss{pblk}_{h}", tag="ip")
                        for h in range(2)
                    ]
                    for half in range(2):
                        for k in range(HT):
                            nc.tensor.matmul(
                                mu_ps[half], ones_sb,
                                pyb[k][:, half * 512 : (half + 1) * 512],
                                start=(k == 0), stop=(k == HT - 1))
                        for k in range(HT):
                            nc.tensor.matmul(
                                ss_ps[half], ones_sb,
                                py2[k][:, half * 512 : (half + 1) * 512],
                                start=(k == 0), stop=(k == HT - 1))
                    pend_stats = (pblk, pyb, mu_ps, ss_ps)
                    pend = None

                if t + 1 < T:
                    xt_nxt = load_x(t + 1)

                # -- close r groups with the recurrent part --
                for k in range(HT):
                    for j in range(4):
                        nc.tensor.matmul(
                            r_ps[j], whh_sb[k][:, j * 128 : (j + 1) * 128],
                            ssl(s_cur, k), start=False, stop=(k == HT - 1))
                # -- gh_n (whh only, complete groups) --
                gh_ps = [
                    ip.tile([128, 512], F32, name=f"gh{t}_{j}", tag="ip")
                    for j in range(2)
                ]
                for k4 in range(4):
                    j = 8 + k4
                    oap = gh_ps[k4 // 2][:, (k4 % 2) * 256 : (k4 % 2) * 256 + 256]
                    for k in range(HT):
                        nc.tensor.matmul(
                            oap, whh_sb[k][:, j * 128 : (j + 1) * 128], ssl(s_cur, k),
                            start=(k == 0), stop=(k == HT - 1))
                # -- z gate (whh + wih complete groups, in-step) --
                z_ps = [
                    ip.tile([128, 512], F32, name=f"z{t}_{j}", tag="ip")
                    for j in range(2)
                ]
                for j4 in range(4):
                    j = 4 + j4
                    oap = z_ps[j4 // 2][:, (j4 % 2) * 256 : (j4 % 2) * 256 + 256]
                    for k in range(HT):
                        nc.tensor.matmul(
                            oap, whh_sb[k][:, j * 128 : (j + 1) * 128], ssl(s_cur, k),
                            start=(k == 0), stop=False)
                    for k in range(HT):
                        nc.tensor.matmul(
                            oap, wih_sb[k][:, j * 128 : (j + 1) * 128], xsl(xt, k),
                            start=False, stop=(k == HT - 1))

                # -- prefill next step (PE stays busy during gate math) --
                if t + 1 < T:
                    gx_ps_nxt = prefill_gxn(t + 1, xt_nxt)
                    r_nxt = prefill_r(t + 1, xt_nxt)

                # -- sigmoids straight from PSUM (bias = b_ih + b_hh) --
                r_t, z_t = [], []
                for k in range(HT):
                    rt = sb.tile([128, S], F16, name=f"rt{t}_{k}", tag="rt", bufs=4)
                    nc.scalar.activation(
                        out=rt, in_=r_ps[k],
                        func=AF.Sigmoid, bias=brz_sb[:, k : k + 1], scale=1.0)
                    r_t.append(rt)
                zt2 = [
                    sb.tile([128, 2 * S], F16, name=f"zt{t}_{p}", tag="zt", bufs=4)
                    for p in range(2)
                ]
                for k in range(HT):
                    j = 4 + k
                    nc.scalar.activation(
                        out=zt2[k // 2][:, (k % 2) * S : (k % 2) * S + S],
                        in_=z_ps[k // 2][:, (k % 2) * 256 : (k % 2) * 256 + 256],
                        func=AF.Sigmoid, bias=brz_sb[:, j : j + 1], scale=1.0)
                # -- n gate --
                st2 = [
                    sb.tile([128, 2 * S], F16, name=f"st{t}_{p}", tag="stt", bufs=4)
                    for p in range(2)
                ]
                for k in range(HT):
                    nc.vector.scalar_tensor_tensor(
                        out=st2[k // 2][:, (k % 2) * S : (k % 2) * S + S],
                        in0=gh_ps[k // 2][:, (k % 2) * 256 : (k % 2) * 256 + 256],
                        scalar=bhn_sb[:, k : k + 1], in1=r_t[k],
                        op0=ALU.add, op1=ALU.mult)
                nt2, hn2 = [], []
                for p in range(2):
                    u = sb.tile([128, 2 * S], F16, name=f"u{t}_{p}", tag="u", bufs=4)
                    nc.vector.tensor_add(u, st2[p], gxs2[p])
                    nt = sb.tile([128, 2 * S], F16, name=f"nt{t}_{p}", tag="nt", bufs=4)
                    nc.scalar.activation(
                        out=nt, in_=u, func=AF.Tanh, scale=1.0)
                    nt2.append(nt)
                # -- hidden update: hn = (s - n)*z + n --
                for p in range(2):
                    t1 = sb.tile([128, 2 * S], F16, name=f"t1{t}_{p}", tag="t1", bufs=4)
                    nc.vector.tensor_sub(t1, s_cur[:, p * 2 * S : (p + 1) * 2 * S], nt2[p])
                    t2 = sb.tile([128, 2 * S], F16, name=f"t2{t}_{p}", tag="t2", bufs=4)
                    nc.vector.tensor_mul(t2, t1, zt2[p])
                    hh = sb.tile([128, 2 * S], F16, name=f"hn{t}_{p}", tag="hn", bufs=4)
                    nc.vector.tensor_add(hh, t2, nt2[p])
                    hn2.append(hh)
                hn = [hn2[k // 2][:, (k % 2) * S : (k % 2) * S + S] for k in range(HT)]

                # -- residual into LN block buffer --
                if main:
                    if toff == 0:
                        y_blk = [
                            sb.tile([128, BLK * S], F16, name=f"yb{blk}_{k}",
                                    tag=f"yb{k}", bufs=2)
                            for k in range(HT)
                        ]
                        y2 = [
                            sb.tile([128, BLK * S], F16, name=f"y2_{blk}_{k}",
                                    tag=f"y2{k}", bufs=2)
                            for k in range(HT)
                        ]
                    for k in range(HT):
                        ysl = y_blk[k][:, toff * S : (toff + 1) * S]
                        yeng = nc.gpsimd if k % 2 else nc.vector
                        yeng.tensor_add(ysl, hn[k], xsl(xt, k))
                        nc.gpsimd.tensor_mul(
                            y2[k][:, toff * S : (toff + 1) * S], ysl, ysl)
                    if toff == BLK - 1:
                        pend = (blk, y_blk, y2)

                # -- next state (masked), h0 injection at entry to main --
                if t + 1 < T:
                    s_nxt = sb.tile([128, HT * S], F16, name=f"s{t + 1}",
                                    tag="state", bufs=4)
                    for p in range(2):
                        mk2 = msk[:, (t + 1) * 2 * S : (t + 2) * 2 * S]
                        nc.vector.tensor_mul(
                            s_nxt[:, p * 2 * S : (p + 1) * 2 * S], hn2[p], mk2)
                    if t + 1 == R:
                        # chunk-0 columns get the true (masked) h0
                        for k in range(HT):
                            nc.gpsimd.tensor_copy(
                                s_nxt[:, k * S : k * S + S : C],
                                h0m_sb[:, k * NB : (k + 1) * NB])
                    s_cur = s_nxt
                    xt = xt_nxt
                    gx_ps_cur = gx_ps_nxt
                    r_ps = r_nxt

                # -- deferred LN stats chain / apply (emitted at step
                # bottom: in-order engine queues run gate-critical ops first)
                if pend_stats is not None:
                    pblk, pyb, mu_ps, ss_ps = pend_stats
                    FB = BLK * S
                    mu_s = sb.tile([1, FB], F16, name=f"mus{pblk}", tag="mus", bufs=2)
                    ss_s = sb.tile([1, FB], F32, name=f"sss{pblk}", tag="sss", bufs=2)
                    for half in range(2):
                        nc.vector.tensor_scalar_mul(
                            mu_s[:, half * 512 : (half + 1) * 512], mu_ps[half],
                            1.0 / H)
                        nc.vector.tensor_scalar_mul(
                            ss_s[:, half * 512 : (half + 1) * 512], ss_ps[half],
                            1.0 / H)
                    var_s = sb.tile([1, FB], F32, name=f"var{pblk}", tag="vars", bufs=2)
                    nc.vector.scalar_tensor_tensor(
                        out=var_s, in0=mu_s, scalar=-1.0, in1=mu_s,
                        op0=ALU.mult, op1=ALU.mult)
                    nc.vector.tensor_add(var_s, var_s, ss_s)
                    std_s = sb.tile([1, FB], F32, name=f"std{pblk}", tag="stds", bufs=2)
                    nc.scalar.activation(
                        out=std_s, in_=var_s, func=AF.Sqrt, bias=eps_sb, scale=1.0)
                    rst_s = sb.tile([1, FB], F32, name=f"rst{pblk}", tag="rsts", bufs=2)
                    nc.vector.reciprocal_approx_fast(out=rst_s, in_=std_s)
                    rst_h = sb.tile([1, FB], F16, name=f"rsh{pblk}", tag="rsth", bufs=2)
                    nc.vector.tensor_copy(rst_h, rst_s)
                    nc.scalar.dma_start(out=scr[pblk : pblk + 1, 0:1024], in_=mu_s)
                    nc.scalar.dma_start(out=scr[pblk : pblk + 1, 1024:2048], in_=rst_h)
                    mu_bc = sb.tile([128, FB], F16, name=f"mubc{pblk}", tag="mubc", bufs=2)
                    rs_bc = sb.tile([128, FB], F16, name=f"rsbc{pblk}", tag="rsbc", bufs=2)
                    nc.scalar.dma_start(out=mu_bc, in_=_bcast_ap(scr[pblk, 0:1024]))
                    nc.scalar.dma_start(out=rs_bc, in_=_bcast_ap(scr[pblk, 1024:2048]))
                    pend_apply = (pblk, pyb, mu_bc, rs_bc, list(range(HT)))
                    pend_stats = None
                elif pend_apply is not None and (not main or toff == 2):
                    pblk, pyb, mu_bc, rs_bc, ks = pend_apply
                    FB = BLK * S
                    half = ks
                    for k in half:
                        yn = sb.tile([128, FB], F16, name=f"yn{pblk}_{k}", tag="yn", bufs=2)
                        nc.vector.tensor_sub(yn, pyb[k], mu_bc)
                        nc.vector.tensor_mul(yn, yn, rs_bc)
                        if not triv_gb:
                            nc.vector.tensor_scalar(
                                out=yn, in0=yn,
                                scalar1=gam_sb[:, k : k + 1],
                                scalar2=bet_sb[:, k : k + 1],
                                op0=ALU.mult, op1=ALU.add)
                        nc.sync.dma_start(
                            out=out_flat[k][:, pblk * FB : (pblk + 1) * FB], in_=yn)
                    rest = [k for k in ks if k not in half]
                    pend_apply = (pblk, pyb, mu_bc, rs_bc, rest) if rest else None

                # -- final hidden state (chunk C-1 columns) --
                if t == T - 1:
                    hlb = sb.tile([128, HT * NB], F16, name="hlb", tag="hlb", bufs=1)
                    for k in range(HT):
                        base = (k % 2) * S
                        nc.vector.tensor_copy(
                            hlb[:, k * NB : (k + 1) * NB],
                            hn2[k // 2][:, base + C - 1 : base + S : C])
                    for k in range(HT):
                        nc.sync.dma_start(
                            out=hl_d[k, :, :],
                            in_=hlb[:, k * NB : (k + 1) * NB])

            # ---- tail: LN for the last two pending blocks ----
            for tail in range(2):
                if pend is not None:
                    pblk, pyb, py2 = pend
                    FB = BLK * S
                    mu_ps = [
                        ip.tile([1, 512], F32, name=f"mu{pblk}_{h}", tag="ip")
                        for h in range(2)
                    ]
                    ss_ps = [
                        ip.tile([1, 512], F32, name=f"ss{pblk}_{h}", tag="ip")
                        for h in range(2)
                    ]
                    for half in range(2):
                        for k in range(HT):
                            nc.tensor.matmul(
                                mu_ps[half], ones_sb,
                                pyb[k][:, half * 512 : (half + 1) * 512],
                                start=(k == 0), stop=(k == HT - 1))
                        for k in range(HT):
                            nc.tensor.matmul(
                                ss_ps[half], ones_sb,
                                py2[k][:, half * 512 : (half + 1) * 512],
                                start=(k == 0), stop=(k == HT - 1))
                    mu_s = sb.tile([1, FB], F16, name=f"mus{pblk}", tag="mus", bufs=2)
                    ss_s = sb.tile([1, FB], F32, name=f"sss{pblk}", tag="sss", bufs=2)
                    for half in range(2):
                        nc.vector.tensor_scalar_mul(
                            mu_s[:, half * 512 : (half + 1) * 512], mu_ps[half],
                            1.0 / H)
                        nc.vector.tensor_scalar_mul(
                            ss_s[:, half * 512 : (half + 1) * 512], ss_ps[half],
                            1.0 / H)
                    var_s = sb.tile([1, FB], F32, name=f"var{pblk}", tag="vars", bufs=2)
                    nc.vector.scalar_tensor_tensor(
                        out=var_s, in0=mu_s, scalar=-1.0, in1=mu_s,
                        op0=ALU.mult, op1=ALU.mult)
                    nc.vector.tensor_add(var_s, var_s, ss_s)
                    std_s = sb.tile([1, FB], F32, name=f"std{pblk}", tag="stds", bufs=2)
                    nc.scalar.activation(
                        out=std_s, in_=var_s, func=AF.Sqrt, bias=eps_sb, scale=1.0)
                    rst_s = sb.tile([1, FB], F32, name=f"rst{pblk}", tag="rsts", bufs=2)
                    nc.vector.reciprocal_approx_fast(out=rst_s, in_=std_s)
                    rst_h = sb.tile([1, FB], F16, name=f"rsh{pblk}", tag="rsth", bufs=2)
                    nc.vector.tensor_copy(rst_h, rst_s)
                    nc.scalar.dma_start(out=scr[pblk : pblk + 1, 0:1024], in_=mu_s)
                    nc.scalar.dma_start(out=scr[pblk : pblk + 1, 1024:2048], in_=rst_h)
                    mu_bc = sb.tile([128, FB], F16, name=f"mubc{pblk}", tag="mubc", bufs=2)
                    rs_bc = sb.tile([128, FB], F16, name=f"rsbc{pblk}", tag="rsbc", bufs=2)
                    nc.scalar.dma_start(out=mu_bc, in_=_bcast_ap(scr[pblk, 0:1024]))
                    nc.scalar.dma_start(out=rs_bc, in_=_bcast_ap(scr[pblk, 1024:2048]))
                    pend = None
                    pend2 = (pblk, pyb, mu_bc, rs_bc, list(range(HT)))
                if pend_apply is not None:
                    pblk, pyb, mu_bc, rs_bc, ks = pend_apply
                    FB = BLK * S
                    for k in ks:
                        yn = sb.tile([128, FB], F16, name=f"yn{pblk}_{k}", tag="yn", bufs=2)
                        nc.vector.tensor_sub(yn, pyb[k], mu_bc)
                        nc.vector.tensor_mul(yn, yn, rs_bc)
                        if not triv_gb:
                            nc.vector.tensor_scalar(
                                out=yn, in0=yn,
                                scalar1=gam_sb[:, k : k + 1],
                                scalar2=bet_sb[:, k : k + 1],
                                op0=ALU.mult, op1=ALU.add)
                        nc.sync.dma_start(
                            out=out_flat[k][:, pblk * FB : (pblk + 1) * FB], in_=yn)
                    pend_apply = None
                if tail == 0:
                    pend_apply = pend2
    nc.compile()
    return nc


def stage_inputs(input, h, is_initial, W_ih, W_hh, b_ih, b_hh, gamma, beta, R):
    """Host-side sharding/staging. Returns per-core input maps."""
    T = R + KS
    x = np.asarray(input, np.float16)
    h0 = np.asarray(h, np.float32)
    ii = np.asarray(is_initial).reshape(N, L)
    W_ih = np.asarray(W_ih, np.float32)
    W_hh = np.asarray(W_hh, np.float32)
    b_ih = np.asarray(b_ih, np.float32)
    b_hh = np.asarray(b_hh, np.float32)

    mask = (1.0 - ii).astype(np.float16)  # [N, L]

    # l index per (c, t): warm-up reads the R steps before the chunk;
    # chunk 0's warm-up reads l in [KS-R, KS) (discarded garbage).
    l_for = np.empty((C, T), np.int64)
    for c in range(C):
        for t in range(T):
            l = c * KS + (t - R)
            l_for[c, t] = l if l >= 0 else l + KS

    wihT = np.ascontiguousarray(
        W_ih.T.reshape(HT, 128, 3 * H)).astype(np.float16)
    whhT = np.ascontiguousarray(
        W_hh.T.reshape(HT, 128, 3 * H)).astype(np.float16)
    brz = (b_ih + b_hh)[: 2 * H].reshape(8, 128).T.copy()        # [128, 8]
    bhn = b_hh[2 * H :].reshape(HT, 128).T.copy()                # [128, 4]
    binn = b_ih[2 * H :].reshape(HT, 128).T.copy()
    gam = np.asarray(gamma, np.float32).reshape(HT, 128).T.copy()
    bet = np.asarray(beta, np.float32).reshape(HT, 128).T.copy()
    ones = np.ones((128, 1), np.float16)

    in_maps = []
    for core in range(NCORES):
        n0 = core * NB
        xc = x[n0 : n0 + NB]              # [NB, L, H]
        # xs[t][p, k*S + s] = x[n, l_for[c, t], k*128+p], s = n*C + c
        xg = xc[:, l_for, :]              # [NB, C, T, H]
        xs = np.ascontiguousarray(
            xg.transpose(2, 3, 0, 1).reshape(T, HT, 128, S)
            .transpose(0, 2, 1, 3).reshape(T, 128, HT * S))
        mg = mask[n0 : n0 + NB][:, l_for]  # [NB, C, T]
        msf = mg.transpose(2, 0, 1).reshape(T, 1, S)
        ms = np.ascontiguousarray(
            np.broadcast_to(msf, (T, 2, S)).reshape(1, T * 2 * S))
        m0 = mask[n0 : n0 + NB, 0].astype(np.float32)  # [NB]
        h0m = np.ascontiguousarray(
            (h0[n0 : n0 + NB] * m0[:, None]).T.reshape(HT, 128, NB)
            .transpose(1, 0, 2).reshape(128, HT * NB)).astype(np.float16)
        in_maps.append({
            "xs": xs, "ms": ms, "h0m": h0m,
            "wih": wihT, "whh": whhT, "brz": brz, "bhn": bhn, "bin": binn,
            "gam": gam, "bet": bet, "ones": ones,
        })
    return in_maps


def required_warmup(is_initial):
    """Max distance from a chunk boundary back to the latest reset."""
    ii = np.asarray(is_initial).reshape(N, L)
    need = 0
    for c in range(1, C):
        start = c * KS
        sub = ii[:, :start]
        for n in range(N):
            nz = np.nonzero(sub[n])[0]
            gap = start - nz[-1] if len(nz) else start
            need = max(need, gap)
    return need


def unstage_outputs(results):
    out = np.empty((N, L, H), np.float32)
    h_last = np.empty((N, H), np.float32)
    for core in range(NCORES):
        n0 = core * NB
        st = results[core]["out_st"].astype(np.float32)  # [HT, 128, KS, S]
        o = st.reshape(HT, 128, KS, NB, C).transpose(3, 4, 2, 0, 1)
        out[n0 : n0 + NB] = o.reshape(NB, L, H)
        hl = results[core]["hlast"].astype(np.float32)  # [HT, 128, NB]
        h_last[n0 : n0 + NB] = hl.transpose(2, 0, 1).reshape(NB, H)
    h_exp = np.broadcast_to(h_last[:, None, :], (N, L, H)).copy()
    return out, h_exp


_PROGRAM_CACHE = {}


def kernel(input, h, is_initial, W_ih, W_hh, b_ih, b_hh, gamma, beta):
    R = max(required_warmup(is_initial), 1)
    triv = bool(
        np.all(np.asarray(gamma) == 1.0) and np.all(np.asarray(beta) == 0.0))
    key = (R, triv)
    if key not in _PROGRAM_CACHE:
        _PROGRAM_CACHE[key] = build_program(R, triv_gb=triv)
    nc = _PROGRAM_CACHE[key]
    in_maps = stage_inputs(
        input, h, is_initial, W_ih, W_hh, b_ih, b_hh, gamma, beta, R)
    res = run_bass_kernel_spmd(nc, in_maps, list(range(NCORES))).results
    return unstage_outputs(res)
